# revision 34
# baseline (speedup 1.0000x reference)
# Multi-head attention kernel for Trainium2 (8 NeuronCores, SPMD).
#
# Problem (hardcoded): X[4, 2048, 1024], W_k/W_q/W_v/W_u[1024, 1024], b_u[1024]
#   K = (X @ W_k.T) * s ; Q = (X @ W_q.T) * s ; V = (X @ W_v.T) * s   (s = 1024**-0.25)
#   S = Q @ K.T per head (16 heads, head_dim 64); P = softmax(S); Y = P @ V
#   out = Y @ W_u.T + b_u
#
# Sharding: core c handles (batch c//2, head-group c%2) -- 8 of the 16 heads.
# Each core computes K/Q/V projections only for its own 8 heads (512 of the
# 1024 features), runs attention for those heads over the full sequence, and
# produces a PARTIAL output projection out_c = (Y_c / D_c) @ W_u.T[gc] + b_u/2.
# The host unshard sums the two partial outputs per batch (pure reduction).
#
# Per-core layout (PE always contracts on partitions):
#   X^T   [e, t]        bf16 from host
#   K^T   [128, j, t]   pair-major: pair j's heads at partitions 0-63 / 64-127
#   Q^T   [128, j, t]   same; lets the two heads of a pair run as CONCURRENT
#                       row-tiled score matmuls (K=64 each, tile (0,0)+(64,0))
#   V     [t, i, h, 65] token-major, 65th column = ones (softmax denominator
#                       arrives free as row 64 of the P@V accumulation)
#   S^T   [tk, q]       scores transposed; exp is layout-agnostic and AV wants
#                       P with keys on partitions
#   Y^T   [128, kt, q]  normalized AV output, bf16, feeds the out-projection
#
# Scale s is folded into the host-side weight slices; all inputs arrive bf16.

import numpy as np
import ml_dtypes

import concourse.bacc as bacc
import concourse.mybir as mybir
import concourse.tile as tile
from concourse.bass_utils import run_bass_kernel_spmd

FP32 = mybir.dt.float32
BF16 = mybir.dt.bfloat16
AF = mybir.ActivationFunctionType

P = 128
E = 1024          # embedding dim
F = 512           # features per core (8 heads x 64)
H = 8             # heads per core
S = 64            # head dim
ET = E // P       # 8 contraction tiles over e
FT = F // P       # 4 feature tiles (= head pairs)
T = 2048          # sequence length
TT = T // P       # 16 key tiles
NQH = 4           # query quarters
QW = T // NQH     # 512 queries per quarter
SCALE = float(1024.0 ** -0.25)

N_CORES = 8


def build_nc():
    nc = bacc.Bacc("TRN2", target_bir_lowering=False, debug=False,
                   enable_asserts=False)

    xt = nc.dram_tensor("xt", [E, T], BF16, kind="ExternalInput").ap()
    wk = nc.dram_tensor("wk", [E, F], BF16, kind="ExternalInput").ap()
    wq = nc.dram_tensor("wq", [E, F], BF16, kind="ExternalInput").ap()
    wv = nc.dram_tensor("wv", [E, F], BF16, kind="ExternalInput").ap()
    wu = nc.dram_tensor("wu", [F, E], BF16, kind="ExternalInput").ap()
    bu = nc.dram_tensor("bu", [1, E], FP32, kind="ExternalInput").ap()
    out = nc.dram_tensor("out", [T, E], BF16, kind="ExternalOutput").ap()

    with tile.TileContext(nc) as tc:
        _build_kernel(tc, nc, xt, wk, wq, wv, wu, bu, out)
    nc.compile()
    return nc


def _build_kernel(tc, nc, xt, wk, wq, wv, wu, bu, out):
    with (
        tc.tile_pool(name="main", bufs=1) as mp,
        tc.tile_pool(name="psum", bufs=1, space="PSUM") as pspool,
        tc.tile_pool(name="dram", bufs=1, space="DRAM") as drampool,
    ):
        # inputs, bf16, tiled for contraction on partitions. Each dma_start
        # costs ~670 ns of serial sync-engine issue time, so batch into as few
        # calls as possible, ordered by first use.
        xtb = mp.tile([P, ET, T], BF16, tag="xtb", name="xtb")
        wkb = mp.tile([P, ET, F], BF16, tag="wkb", name="wkb")
        wqb = mp.tile([P, ET, F], BF16, tag="wqb", name="wqb")
        wvb = mp.tile([P, ET, F], BF16, tag="wvb", name="wvb")
        nc.sync.dma_start(wkb[:], wk.rearrange("(k p) f -> p k f", p=P))
        nc.sync.dma_start(wqb[:], wq.rearrange("(k p) f -> p k f", p=P))
        for k in range(ET):
            nc.sync.dma_start(xtb[:, k, :], xt[k * P:(k + 1) * P, :])
            if k == 3:
                nc.sync.dma_start(
                    wvb[:], wv.rearrange("(k p) f -> p k f", p=P))
        bub = mp.tile([P, E], FP32, tag="bub", name="bub")
        nc.sync.dma_start(bub[:], bu.to_broadcast([P, E]))

        ktb = mp.tile([P, FT, T], BF16, tag="ktb", name="ktb")
        qtb = mp.tile([P, FT, T], BF16, tag="qtb", name="qtb")
        vv = mp.tile([P, TT, H, S + 1], BF16, tag="vv", name="vv")
        yt = mp.tile([P, FT, T], BF16, tag="yt", name="yt")

        def kq_piece(j, wb, dst, c0, half, state):
            """Half of one 512-column K/Q projection chunk (4 of 8 k-tiles);
            the accumulation group spans both pieces so a piece fits in the
            attention loop's per-slot PE slack."""
            if half == 0:
                state[(j, id(wb), c0)] = pspool.tile(
                    [P, 512], FP32, tag="fps", bufs=2,
                    name=f"pskq{j}_{id(wb)}_{c0}")
            ps = state[(j, id(wb), c0)]
            for k in range(half * 2, half * 2 + 2):
                nc.tensor.matmul(
                    ps[:],
                    lhsT=wb[:, k, j * P:(j + 1) * P],
                    rhs=xtb[:, k, c0:c0 + 512],
                    start=(k == 0), stop=(k == ET - 1))
            if half == 3:
                del state[(j, id(wb), c0)]
                nc.vector.tensor_copy(out=dst[:, j, c0:c0 + 512],
                                      in_=ps[:, 0:512])

        def v_group(mt):
            """V projection for token tile mt -> vv[:, mt] + ones column."""
            ps = pspool.tile([P, 512], FP32, tag="fps", bufs=2,
                             name=f"psv{mt}")
            for k in range(ET):
                nc.tensor.matmul(
                    ps[:],
                    lhsT=xtb[:, k, mt * P:(mt + 1) * P],
                    rhs=wvb[:, k, :],
                    start=(k == 0), stop=(k == ET - 1))
            nc.vector.tensor_copy(
                out=vv[:, mt, :, 0:S],
                in_=ps[:].rearrange("p (h s) -> p h s", s=S))
            nc.vector.memset(vv[:, mt, :, S:S + 1], 1.0)

        # K/Q projection of pair 0 first (fills the score pipeline), then a
        # head start on V so AV never outruns the V projection.
        # touch the score-psum tag first so its two [128,1024] buffers land
        # at PSUM banks 0-3 (exp reads alternate between the two buffers)
        pspool.tile([P, 1024], FP32, tag="ps", bufs=2, name="ps_pin0")
        pspool.tile([P, 1024], FP32, tag="ps", bufs=2, name="ps_pin1")
        _st = {}
        for c0 in range(0, T, 512):
            for half in range(4):
                kq_piece(0, wkb, ktb, c0, half, _st)
        for half in range(4):
            kq_piece(0, wqb, qtb, 0, half, _st)
        for mt in range(3):
            v_group(mt)

        # wu shares wvb's slot (dead after V projection completes)
        wub = mp.tile([P, FT, E], BF16, tag="wvb", name="wub")

        def load_wu():
            nc.sync.dma_start(
                wub[:], wu.rearrange("(kt p) e -> p kt e", p=P))

        # per-(pair, slot) filler work, spread into the attention loop's PE
        # slack so the scalar engine (exp) stays the critical path. V tiles
        # must land at >= 1/slot during pair 0's first quarter (AV consumes
        # V(i) at slot i), so they occupy slots 0..12 back-to-back.
        sched = {j: {} for j in range(FT)}
        for idx, mt in enumerate(range(3, TT)):
            sched[0].setdefault(idx, []).append(lambda mt=mt: v_group(mt))
        for qh in range(NQH - 1):
            c0 = (qh + 1) * QW
            for half in range(4):
                sched[0].setdefault(qh * TT + 11 + half, []).append(
                    lambda c0=c0, half=half:
                    kq_piece(0, wqb, qtb, c0, half, kq_state))

        kq_state = {}

        def kq_closures(nj):
            return [
                (lambda nj=nj, wb=wb, dst=dst, c0=c0, half=half:
                 kq_piece(nj, wb, dst, c0, half, kq_state))
                for wb, dst in ((wkb, ktb), (wqb, qtb))
                for c0 in range(0, T, 512)
                for half in range(4)
            ]

        rest0 = kq_closures(1) + [load_wu]
        for idx, fn in enumerate(rest0):
            s = 13 + (NQH * TT - 13) * idx // len(rest0)
            sched[0].setdefault(s, []).append(fn)
        for j in (1, 2):
            lst = kq_closures(j + 1)
            for idx, fn in enumerate(lst):
                sched[j].setdefault(NQH * TT * idx // len(lst), []).append(fn)
        for qh in range(NQH - 1):
            for idx, (m, n0) in enumerate(
                    [(m, n0) for m in range(QW // P) for n0 in (0, 512)]):
                sched[FT - 1].setdefault(
                    (qh + 1) * TT + 1 + idx * 14 // 8, []).append(
                    lambda qh=qh, m=m, n0=n0: epi_outproj_piece(qh, m, n0))

        # epilogue state: unnormalized Y (bf16) + denominators staged in DRAM
        yraws = {}
        dramd = {qh: drampool.tile([H, QW], BF16, tag=f"dD{qh}", bufs=1,
                                   name=f"dramD{qh}")
                 for qh in range(NQH)}

        def epi_norm(qh, j_lo=0, j_hi=FT):
            """Batched reciprocal of denominators for pairs [j_lo, j_hi) of
            this quarter, broadcast via DRAM bounce, normalize into yt (DVE +
            DMA only). The last quarter runs pairs 0-2 early (their AV drains
            finish during pair 3's attention) so only pair 3's two heads sit
            on the serial tail."""
            q0 = qh * QW
            h0, nh = 2 * j_lo, 2 * (j_hi - j_lo)
            dsb = mp.tile([P, QW], BF16, tag="dsb", bufs=2,
                          name=f"dsb{qh}_{j_lo}")
            nc.sync.dma_start(dsb[0:nh, :], dramd[qh][h0:h0 + nh, :])
            rcb = mp.tile([P, QW], FP32, tag="rcb", bufs=2,
                          name=f"rcb{qh}_{j_lo}")
            nc.vector.reciprocal(out=rcb[0:nh, :], in_=dsb[0:nh, :])
            dramr = drampool.tile([H, QW], FP32, tag="dR", bufs=2,
                                  name=f"dramR{qh}_{j_lo}")
            nc.sync.dma_start(dramr[0:nh, :], rcb[0:nh, :])
            for j in range(j_lo, j_hi):
                for par in range(2):
                    h = 2 * j + par
                    hl = h - h0
                    rbc = mp.tile([S, QW], FP32, tag="rbc", bufs=4,
                                  name=f"rbc{qh}_{h}")
                    nc.sync.dma_start(
                        rbc[:], dramr[hl:hl + 1, :].to_broadcast([S, QW]))
                    yraw = yraws.pop((qh, j, par))
                    if par == 0:
                        nc.vector.tensor_mul(out=yt[0:S, j, q0:q0 + QW],
                                             in0=yraw[0:S, :], in1=rbc[:])
                    else:
                        tmp = mp.tile([S, QW], BF16, tag="tmp", bufs=2,
                                      name=f"tmp{qh}_{h}")
                        nc.vector.tensor_mul(out=tmp[:], in0=yraw[0:S, :],
                                             in1=rbc[:])
                        nc.sync.dma_start(yt[S:P, j, q0:q0 + QW], tmp[:])

        otiles = {}

        def epi_outproj_piece(qh, m, n0):
            """One [128, 512] chunk of the output projection for quarter qh;
            the two n0-halves of an m-tile share one staging tile and one
            output DMA (sync-engine DMA issues are ~670 ns each)."""
            mc = qh * QW + m * P
            ps = pspool.tile([P, 512], FP32, tag="fps", bufs=2,
                             name=f"pso{qh}_{m}_{n0}")
            for kt in range(FT):
                nc.tensor.matmul(
                    ps[:],
                    lhsT=yt[:, kt, mc:mc + P],
                    rhs=wub[:, kt, n0:n0 + 512],
                    start=(kt == 0), stop=(kt == FT - 1))
            if n0 == 0:
                otiles[(qh, m)] = mp.tile([P, E], BF16, tag="ot", bufs=3,
                                          name=f"ot{qh}_{m}")
            ot = otiles[(qh, m)]
            nc.vector.tensor_add(out=ot[:, n0:n0 + 512], in0=ps[:],
                                 in1=bub[:, n0:n0 + 512])
            if n0 == 512:
                del otiles[(qh, m)]
                nc.sync.dma_start(out[mc:mc + P, :], ot[:])

        # --- attention: pair-outer, query-quarter inner ---
        for j in range(FT):
            fill = sched[j]
            slot = 0
            for qh in range(NQH):
                q0 = qh * QW
                avA = pspool.tile([P, QW], FP32, tag="avA", bufs=1,
                                  name=f"av{j}_{qh}_0")
                avB = pspool.tile([P, QW], FP32, tag="avB", bufs=1,
                                  name=f"av{j}_{qh}_1")
                def emit_av(i, pts):
                    nc.tensor.matmul(
                        avA[0:S + 1, :],
                        lhsT=vv[:, i, 2 * j, :],
                        rhs=pts[:, 0:512],
                        start=(i == 0), stop=(i == TT - 1))
                    nc.tensor.matmul(
                        avB[0:S + 1, :],
                        lhsT=vv[:, i, 2 * j + 1, :],
                        rhs=pts[:, 512:1024],
                        start=(i == 0), stop=(i == TT - 1))

                # AV runs two iterations behind the scores/exp so the PE never
                # sem-waits on the scalar engine inside its queue
                pend = []
                for i in range(TT):
                    ps = pspool.tile([P, 1024], FP32, tag="ps", bufs=2,
                                     name=f"s{j}_{qh}_{i}")
                    # two heads of the pair: concurrent row-tiled matmuls
                    nc.tensor.matmul(
                        ps[:, 0:512],
                        lhsT=ktb[0:S, j, i * P:(i + 1) * P],
                        rhs=qtb[0:S, j, q0:q0 + QW],
                        start=True, stop=True)
                    nc.tensor.matmul(
                        ps[:, 512:1024],
                        lhsT=ktb[S:P, j, i * P:(i + 1) * P],
                        rhs=qtb[S:P, j, q0:q0 + QW],
                        start=True, stop=True)
                    pts = mp.tile([P, 1024], BF16, tag="pt", bufs=6,
                                  name=f"p{j}_{qh}_{i}")
                    nc.scalar.activation(pts[:], ps[:], AF.Exp)
                    pend.append((i, pts))
                    if len(pend) > 3:
                        emit_av(*pend.pop(0))
                    # spread filler projections evenly over this pair's slots
                    for fn in fill.get(slot, ()):
                        fn()
                    slot += 1
                for item in pend:
                    emit_av(*item)
                # drain AV into bf16 staging; denominator row -> DRAM
                for par, av in ((0, avA), (1, avB)):
                    yraw = mp.tile([P, QW], BF16, tag="yraw", bufs=34,
                                   name=f"yraw{j}_{qh}_{par}")
                    if j == FT - 1 and qh == NQH - 1 and par == 1:
                        nc.scalar.copy(out=yraw[0:S + 1, :],
                                       in_=av[0:S + 1, :])
                    else:
                        nc.vector.tensor_copy(out=yraw[0:S + 1, :],
                                              in_=av[0:S + 1, :])
                    nc.sync.dma_start(dramd[qh][2 * j + par:2 * j + par + 1, :],
                                      yraw[S:S + 1, :])
                    yraws[(qh, j, par)] = yraw
                if j == FT - 1 and qh < NQH - 1:
                    epi_norm(qh)
        epi_norm(NQH - 1)
        for m in range(QW // P):
            for n0 in (0, 512):
                epi_outproj_piece(NQH - 1, m, n0)


_NC = None


def _get_nc():
    global _NC
    if _NC is None:
        _NC = build_nc()
    return _NC


def make_in_maps(X, W_k, W_q, W_v, W_u, b_u):
    bf16 = ml_dtypes.bfloat16
    X = np.asarray(X, np.float32)
    b = X.shape[0]
    wkt = (np.asarray(W_k, np.float32).T * SCALE).astype(bf16)
    wqt = (np.asarray(W_q, np.float32).T * SCALE).astype(bf16)
    wvt = (np.asarray(W_v, np.float32).T * SCALE).astype(bf16)
    wut = np.ascontiguousarray(np.asarray(W_u, np.float32).T).astype(bf16)
    bu2 = np.ascontiguousarray(
        (np.asarray(b_u, np.float32) * 0.5).reshape(1, E))
    xts = [np.ascontiguousarray(X[bi].T).astype(bf16) for bi in range(b)]
    in_maps = []
    for c in range(N_CORES):
        bi, pg = c // 2, c % 2
        f0 = pg * F
        in_maps.append({
            "xt": xts[bi],
            "wk": np.ascontiguousarray(wkt[:, f0:f0 + F]),
            "wq": np.ascontiguousarray(wqt[:, f0:f0 + F]),
            "wv": np.ascontiguousarray(wvt[:, f0:f0 + F]),
            "wu": np.ascontiguousarray(wut[f0:f0 + F, :]),
            "bu": bu2,
        })
    return in_maps


def run(inputs, trace=False, **kwargs):
    """Run on hardware; returns (full output, BassKernelResults)."""
    X = np.asarray(inputs["X"], np.float32)
    b, t, e = X.shape
    nc = _get_nc()
    in_maps = make_in_maps(X, inputs["W_k"], inputs["W_q"], inputs["W_v"],
                           inputs["W_u"], inputs["b_u"])
    res = run_bass_kernel_spmd(nc, in_maps, core_ids=list(range(N_CORES)),
                               trace=trace, **kwargs)
    full = np.empty((b, t, e), np.float32)
    for bi in range(b):
        full[bi] = (res.results[2 * bi]["out"].astype(np.float32)
                    + res.results[2 * bi + 1]["out"].astype(np.float32))
    return full, res


def kernel(**inputs):
    full, _ = run(inputs)
    return full


# revision 35
# speedup vs baseline: 1.0244x; 1.0244x over previous
# Multi-head attention kernel for Trainium2 (8 NeuronCores, SPMD).
#
# Problem (hardcoded): X[4, 2048, 1024], W_k/W_q/W_v/W_u[1024, 1024], b_u[1024]
#   K = (X @ W_k.T) * s ; Q = (X @ W_q.T) * s ; V = (X @ W_v.T) * s   (s = 1024**-0.25)
#   S = Q @ K.T per head (16 heads, head_dim 64); P = softmax(S); Y = P @ V
#   out = Y @ W_u.T + b_u
#
# Sharding: core c handles (batch c//2, head-group c%2) -- 8 of the 16 heads.
# Each core computes K/Q/V projections only for its own 8 heads (512 of the
# 1024 features), runs attention for those heads over the full sequence, and
# produces a PARTIAL output projection out_c = (Y_c / D_c) @ W_u.T[gc] + b_u/2.
# The host unshard sums the two partial outputs per batch (pure reduction).
#
# Per-core layout (PE always contracts on partitions):
#   X^T   [e, t]        bf16 from host
#   K^T   [128, j, t]   pair-major: pair j's heads at partitions 0-63 / 64-127
#   Q^T   [128, j, t]   same; lets the two heads of a pair run as CONCURRENT
#                       row-tiled score matmuls (K=64 each, tile (0,0)+(64,0))
#   V     [t, i, h, 65] token-major, 65th column = ones (softmax denominator
#                       arrives free as row 64 of the P@V accumulation)
#   S^T   [tk, q]       scores transposed; exp is layout-agnostic and AV wants
#                       P with keys on partitions
#   Y^T   [128, kt, q]  normalized AV output, bf16, feeds the out-projection
#
# Scale s is folded into the host-side weight slices; all inputs arrive bf16.

import numpy as np
import ml_dtypes

import concourse.bacc as bacc
import concourse.mybir as mybir
import concourse.tile as tile
from concourse.bass_utils import run_bass_kernel_spmd

FP32 = mybir.dt.float32
BF16 = mybir.dt.bfloat16
AF = mybir.ActivationFunctionType

P = 128
E = 1024          # embedding dim
F = 512           # features per core (8 heads x 64)
H = 8             # heads per core
S = 64            # head dim
ET = E // P       # 8 contraction tiles over e
FT = F // P       # 4 feature tiles (= head pairs)
T = 2048          # sequence length
TT = T // P       # 16 key tiles
NQH = 4           # query quarters
QW = T // NQH     # 512 queries per quarter
SCALE = float(1024.0 ** -0.25)

N_CORES = 8


def build_nc():
    nc = bacc.Bacc("TRN2", target_bir_lowering=False, debug=False,
                   enable_asserts=False)

    xt = nc.dram_tensor("xt", [E, T], BF16, kind="ExternalInput").ap()
    wk = nc.dram_tensor("wk", [E, F], BF16, kind="ExternalInput").ap()
    wq = nc.dram_tensor("wq", [E, F], BF16, kind="ExternalInput").ap()
    wv = nc.dram_tensor("wv", [E, F], BF16, kind="ExternalInput").ap()
    wu = nc.dram_tensor("wu", [F, E], BF16, kind="ExternalInput").ap()
    bu = nc.dram_tensor("bu", [1, E], FP32, kind="ExternalInput").ap()
    out = nc.dram_tensor("out", [T, E], BF16, kind="ExternalOutput").ap()

    with tile.TileContext(nc) as tc:
        _build_kernel(tc, nc, xt, wk, wq, wv, wu, bu, out)
    nc.compile()
    return nc


def _build_kernel(tc, nc, xt, wk, wq, wv, wu, bu, out):
    with (
        tc.tile_pool(name="main", bufs=1) as mp,
        tc.tile_pool(name="psum", bufs=1, space="PSUM") as pspool,
        tc.tile_pool(name="dram", bufs=1, space="DRAM") as drampool,
    ):
        # inputs, bf16, tiled for contraction on partitions. Each dma_start
        # costs ~670 ns of serial sync-engine issue time, so batch into as few
        # calls as possible, ordered by first use.
        xtb = mp.tile([P, ET, T], BF16, tag="xtb", name="xtb")
        wkb = mp.tile([P, ET, F], BF16, tag="wkb", name="wkb")
        wqb = mp.tile([P, ET, F], BF16, tag="wqb", name="wqb")
        wvb = mp.tile([P, ET, F], BF16, tag="wvb", name="wvb")
        nc.sync.dma_start(wkb[:], wk.rearrange("(k p) f -> p k f", p=P))
        nc.scalar.dma_start(wqb[:], wq.rearrange("(k p) f -> p k f", p=P))
        for k in range(ET):
            eng = nc.sync if k % 2 == 0 else nc.scalar
            eng.dma_start(xtb[:, k, :], xt[k * P:(k + 1) * P, :])
            if k == 3:
                nc.scalar.dma_start(
                    wvb[:], wv.rearrange("(k p) f -> p k f", p=P))
        bub = mp.tile([P, E], FP32, tag="bub", name="bub")
        nc.sync.dma_start(bub[:], bu.to_broadcast([P, E]))

        ktb = mp.tile([P, FT, T], BF16, tag="ktb", name="ktb")
        qtb = mp.tile([P, FT, T], BF16, tag="qtb", name="qtb")
        vv = mp.tile([P, TT, H, S + 1], BF16, tag="vv", name="vv")
        yt = mp.tile([P, FT, T], BF16, tag="yt", name="yt")

        def kq_piece(j, wb, dst, c0, half, state):
            """Half of one 512-column K/Q projection chunk (4 of 8 k-tiles);
            the accumulation group spans both pieces so a piece fits in the
            attention loop's per-slot PE slack."""
            if half == 0:
                state[(j, id(wb), c0)] = pspool.tile(
                    [P, 512], FP32, tag="fps", bufs=2,
                    name=f"pskq{j}_{id(wb)}_{c0}")
            ps = state[(j, id(wb), c0)]
            for k in range(half * 2, half * 2 + 2):
                nc.tensor.matmul(
                    ps[:],
                    lhsT=wb[:, k, j * P:(j + 1) * P],
                    rhs=xtb[:, k, c0:c0 + 512],
                    start=(k == 0), stop=(k == ET - 1))
            if half == 3:
                del state[(j, id(wb), c0)]
                nc.vector.tensor_copy(out=dst[:, j, c0:c0 + 512],
                                      in_=ps[:, 0:512])

        def v_group(mt):
            """V projection for token tile mt -> vv[:, mt] + ones column."""
            ps = pspool.tile([P, 512], FP32, tag="fps", bufs=2,
                             name=f"psv{mt}")
            for k in range(ET):
                nc.tensor.matmul(
                    ps[:],
                    lhsT=xtb[:, k, mt * P:(mt + 1) * P],
                    rhs=wvb[:, k, :],
                    start=(k == 0), stop=(k == ET - 1))
            nc.vector.tensor_copy(
                out=vv[:, mt, :, 0:S],
                in_=ps[:].rearrange("p (h s) -> p h s", s=S))
            nc.vector.memset(vv[:, mt, :, S:S + 1], 1.0)

        # K/Q projection of pair 0 first (fills the score pipeline), then a
        # head start on V so AV never outruns the V projection.
        # touch the score-psum tag first so its two [128,1024] buffers land
        # at PSUM banks 0-3 (exp reads alternate between the two buffers)
        pspool.tile([P, 1024], FP32, tag="ps", bufs=2, name="ps_pin0")
        pspool.tile([P, 1024], FP32, tag="ps", bufs=2, name="ps_pin1")
        _st = {}
        for c0 in range(0, T, 512):
            for half in range(4):
                kq_piece(0, wkb, ktb, c0, half, _st)
        for half in range(4):
            kq_piece(0, wqb, qtb, 0, half, _st)
        for mt in range(3):
            v_group(mt)

        # wu shares wvb's slot (dead after V projection completes)
        wub = mp.tile([P, FT, E], BF16, tag="wvb", name="wub")

        def load_wu():
            nc.sync.dma_start(
                wub[:], wu.rearrange("(kt p) e -> p kt e", p=P))

        # per-(pair, slot) filler work, spread into the attention loop's PE
        # slack so the scalar engine (exp) stays the critical path. V tiles
        # must land at >= 1/slot during pair 0's first quarter (AV consumes
        # V(i) at slot i), so they occupy slots 0..12 back-to-back.
        sched = {j: {} for j in range(FT)}
        for idx, mt in enumerate(range(3, TT)):
            sched[0].setdefault(idx, []).append(lambda mt=mt: v_group(mt))
        for qh in range(NQH - 1):
            c0 = (qh + 1) * QW
            for half in range(4):
                sched[0].setdefault(qh * TT + 11 + half, []).append(
                    lambda c0=c0, half=half:
                    kq_piece(0, wqb, qtb, c0, half, kq_state))

        kq_state = {}

        def kq_closures(nj):
            return [
                (lambda nj=nj, wb=wb, dst=dst, c0=c0, half=half:
                 kq_piece(nj, wb, dst, c0, half, kq_state))
                for wb, dst in ((wkb, ktb), (wqb, qtb))
                for c0 in range(0, T, 512)
                for half in range(4)
            ]

        rest0 = kq_closures(1) + [load_wu]
        for idx, fn in enumerate(rest0):
            s = 13 + (NQH * TT - 13) * idx // len(rest0)
            sched[0].setdefault(s, []).append(fn)
        for j in (1, 2):
            lst = kq_closures(j + 1)
            for idx, fn in enumerate(lst):
                sched[j].setdefault(NQH * TT * idx // len(lst), []).append(fn)
        for qh in range(NQH - 1):
            for idx, (m, n0) in enumerate(
                    [(m, n0) for m in range(QW // P) for n0 in (0, 512)]):
                sched[FT - 1].setdefault(
                    (qh + 1) * TT + 1 + idx * 14 // 8, []).append(
                    lambda qh=qh, m=m, n0=n0: epi_outproj_piece(qh, m, n0))

        # epilogue state: unnormalized Y (bf16) + denominators staged in DRAM
        yraws = {}
        dramd = {qh: drampool.tile([H, QW], BF16, tag=f"dD{qh}", bufs=1,
                                   name=f"dramD{qh}")
                 for qh in range(NQH)}

        def epi_norm(qh, j_lo=0, j_hi=FT):
            """Batched reciprocal of denominators for pairs [j_lo, j_hi) of
            this quarter, broadcast via DRAM bounce, normalize into yt (DVE +
            DMA only). The last quarter runs pairs 0-2 early (their AV drains
            finish during pair 3's attention) so only pair 3's two heads sit
            on the serial tail."""
            q0 = qh * QW
            de = nc.scalar if qh == NQH - 1 else nc.sync
            h0, nh = 2 * j_lo, 2 * (j_hi - j_lo)
            dsb = mp.tile([P, QW], BF16, tag="dsb", bufs=2,
                          name=f"dsb{qh}_{j_lo}")
            de.dma_start(dsb[0:nh, :], dramd[qh][h0:h0 + nh, :])
            rcb = mp.tile([P, QW], FP32, tag="rcb", bufs=2,
                          name=f"rcb{qh}_{j_lo}")
            nc.vector.reciprocal(out=rcb[0:nh, :], in_=dsb[0:nh, :])
            dramr = drampool.tile([H, QW], FP32, tag="dR", bufs=2,
                                  name=f"dramR{qh}_{j_lo}")
            de.dma_start(dramr[0:nh, :], rcb[0:nh, :])
            for j in range(j_lo, j_hi):
                for par in range(2):
                    h = 2 * j + par
                    hl = h - h0
                    rbc = mp.tile([S, QW], FP32, tag="rbc", bufs=4,
                                  name=f"rbc{qh}_{h}")
                    de.dma_start(
                        rbc[:], dramr[hl:hl + 1, :].to_broadcast([S, QW]))
                    yraw = yraws.pop((qh, j, par))
                    if par == 0:
                        nc.vector.tensor_mul(out=yt[0:S, j, q0:q0 + QW],
                                             in0=yraw[0:S, :], in1=rbc[:])
                    else:
                        tmp = mp.tile([S, QW], BF16, tag="tmp", bufs=2,
                                      name=f"tmp{qh}_{h}")
                        nc.vector.tensor_mul(out=tmp[:], in0=yraw[0:S, :],
                                             in1=rbc[:])
                        de.dma_start(yt[S:P, j, q0:q0 + QW], tmp[:])

        otiles = {}

        def epi_outproj_piece(qh, m, n0):
            """One [128, 512] chunk of the output projection for quarter qh;
            the two n0-halves of an m-tile share one staging tile and one
            output DMA (sync-engine DMA issues are ~670 ns each)."""
            mc = qh * QW + m * P
            ps = pspool.tile([P, 512], FP32, tag="fps", bufs=2,
                             name=f"pso{qh}_{m}_{n0}")
            for kt in range(FT):
                nc.tensor.matmul(
                    ps[:],
                    lhsT=yt[:, kt, mc:mc + P],
                    rhs=wub[:, kt, n0:n0 + 512],
                    start=(kt == 0), stop=(kt == FT - 1))
            if n0 == 0:
                otiles[(qh, m)] = mp.tile([P, E], BF16, tag="ot", bufs=3,
                                          name=f"ot{qh}_{m}")
            ot = otiles[(qh, m)]
            nc.vector.tensor_add(out=ot[:, n0:n0 + 512], in0=ps[:],
                                 in1=bub[:, n0:n0 + 512])
            if n0 == 512:
                del otiles[(qh, m)]
                nc.sync.dma_start(out[mc:mc + P, :], ot[:])

        # --- attention: pair-outer, query-quarter inner ---
        for j in range(FT):
            fill = sched[j]
            slot = 0
            for qh in range(NQH):
                q0 = qh * QW
                avA = pspool.tile([P, QW], FP32, tag="avA", bufs=1,
                                  name=f"av{j}_{qh}_0")
                avB = pspool.tile([P, QW], FP32, tag="avB", bufs=1,
                                  name=f"av{j}_{qh}_1")
                def emit_av(i, pts):
                    nc.tensor.matmul(
                        avA[0:S + 1, :],
                        lhsT=vv[:, i, 2 * j, :],
                        rhs=pts[:, 0:512],
                        start=(i == 0), stop=(i == TT - 1))
                    nc.tensor.matmul(
                        avB[0:S + 1, :],
                        lhsT=vv[:, i, 2 * j + 1, :],
                        rhs=pts[:, 512:1024],
                        start=(i == 0), stop=(i == TT - 1))

                # AV runs two iterations behind the scores/exp so the PE never
                # sem-waits on the scalar engine inside its queue
                pend = []
                for i in range(TT):
                    ps = pspool.tile([P, 1024], FP32, tag="ps", bufs=2,
                                     name=f"s{j}_{qh}_{i}")
                    # two heads of the pair: concurrent row-tiled matmuls
                    nc.tensor.matmul(
                        ps[:, 0:512],
                        lhsT=ktb[0:S, j, i * P:(i + 1) * P],
                        rhs=qtb[0:S, j, q0:q0 + QW],
                        start=True, stop=True)
                    nc.tensor.matmul(
                        ps[:, 512:1024],
                        lhsT=ktb[S:P, j, i * P:(i + 1) * P],
                        rhs=qtb[S:P, j, q0:q0 + QW],
                        start=True, stop=True)
                    pts = mp.tile([P, 1024], BF16, tag="pt", bufs=6,
                                  name=f"p{j}_{qh}_{i}")
                    nc.scalar.activation(pts[:], ps[:], AF.Exp)
                    pend.append((i, pts))
                    if len(pend) > 3:
                        emit_av(*pend.pop(0))
                    # spread filler projections evenly over this pair's slots
                    for fn in fill.get(slot, ()):
                        fn()
                    slot += 1
                for item in pend:
                    emit_av(*item)
                # drain AV into bf16 staging; denominator row -> DRAM
                for par, av in ((0, avA), (1, avB)):
                    yraw = mp.tile([P, QW], BF16, tag="yraw", bufs=34,
                                   name=f"yraw{j}_{qh}_{par}")
                    if j == FT - 1 and qh == NQH - 1 and par == 1:
                        nc.scalar.copy(out=yraw[0:S + 1, :],
                                       in_=av[0:S + 1, :])
                    else:
                        nc.vector.tensor_copy(out=yraw[0:S + 1, :],
                                              in_=av[0:S + 1, :])
                    nc.sync.dma_start(dramd[qh][2 * j + par:2 * j + par + 1, :],
                                      yraw[S:S + 1, :])
                    yraws[(qh, j, par)] = yraw
                if j == FT - 1 and qh < NQH - 1:
                    epi_norm(qh)
        epi_norm(NQH - 1)
        for m in range(QW // P):
            for n0 in (0, 512):
                epi_outproj_piece(NQH - 1, m, n0)


_NC = None


def _get_nc():
    global _NC
    if _NC is None:
        _NC = build_nc()
    return _NC


def make_in_maps(X, W_k, W_q, W_v, W_u, b_u):
    bf16 = ml_dtypes.bfloat16
    X = np.asarray(X, np.float32)
    b = X.shape[0]
    wkt = (np.asarray(W_k, np.float32).T * SCALE).astype(bf16)
    wqt = (np.asarray(W_q, np.float32).T * SCALE).astype(bf16)
    wvt = (np.asarray(W_v, np.float32).T * SCALE).astype(bf16)
    wut = np.ascontiguousarray(np.asarray(W_u, np.float32).T).astype(bf16)
    bu2 = np.ascontiguousarray(
        (np.asarray(b_u, np.float32) * 0.5).reshape(1, E))
    xts = [np.ascontiguousarray(X[bi].T).astype(bf16) for bi in range(b)]
    in_maps = []
    for c in range(N_CORES):
        bi, pg = c // 2, c % 2
        f0 = pg * F
        in_maps.append({
            "xt": xts[bi],
            "wk": np.ascontiguousarray(wkt[:, f0:f0 + F]),
            "wq": np.ascontiguousarray(wqt[:, f0:f0 + F]),
            "wv": np.ascontiguousarray(wvt[:, f0:f0 + F]),
            "wu": np.ascontiguousarray(wut[f0:f0 + F, :]),
            "bu": bu2,
        })
    return in_maps


def run(inputs, trace=False, **kwargs):
    """Run on hardware; returns (full output, BassKernelResults)."""
    X = np.asarray(inputs["X"], np.float32)
    b, t, e = X.shape
    nc = _get_nc()
    in_maps = make_in_maps(X, inputs["W_k"], inputs["W_q"], inputs["W_v"],
                           inputs["W_u"], inputs["b_u"])
    res = run_bass_kernel_spmd(nc, in_maps, core_ids=list(range(N_CORES)),
                               trace=trace, **kwargs)
    full = np.empty((b, t, e), np.float32)
    for bi in range(b):
        full[bi] = (res.results[2 * bi]["out"].astype(np.float32)
                    + res.results[2 * bi + 1]["out"].astype(np.float32))
    return full, res


def kernel(**inputs):
    full, _ = run(inputs)
    return full
